# revision 15
# baseline (speedup 1.0000x reference)
"""Trainium2 Bass kernel for nn_Attention_85074712199827.

Computes, for hidden [1,32,1024], encoder_outputs [32,2048,1024],
W_attn [1024,2048], b_attn [1024], v [1024]:

    h_proj  = hidden[0] @ W_attn[:, :1024].T
    e_proj  = encoder_outputs @ W_attn[:, 1024:].T
    energy  = tanh(e_proj + h_proj[:, None, :] + b_attn)
    att     = energy @ v
    out     = softmax(att, axis=1)          # [32, 2048] float32

Distribution: data-parallel over the batch across 8 NeuronCores (4
batch rows per core); parameters replicated. All operands are pre-laid
out on the host: W_attn.T and hidden.T in bf16, and encoder_outputs
pre-transposed per batch row to [h, s] bf16, so the device needs no
on-chip transposes or casts — the PE consumes DMA-ed tiles directly.

PE stream: a short back-to-back warmup burst (trips the HAM clock gate
to 8/8 while the first DMAs land), then 16 units (4 batch rows x 4
s-chunks of 512) of 64 e_proj matmuls each; h_proj's 64 small matmuls
are spliced into unit 0 after its 6th group so the PE never waits on
the Wh weights. The v-weighted hidden-axis reduction runs on the
vector engine (one fused multiply-add per 128-chunk) plus a single
ones-vector matmul per unit. Softmax uses a shift by the bound
sum(|v|) >= |att| instead of the max, so exp and the normalization sum
run per-unit, fully overlapped; only the last row's normalize+store
remains in the tail.

Self-contained: only environment packages (concourse, numpy, ml_dtypes)
are imported; all shapes/sharding are hardcoded for this problem.
"""

from contextlib import ExitStack

import ml_dtypes
import numpy as np

import concourse.bass as bass  # noqa: F401  (namespace import keeps parity with env)
import concourse.tile as tile
from concourse import bacc, mybir

F32 = mybir.dt.float32
BF16 = mybir.dt.bfloat16
AF = mybir.ActivationFunctionType
ALU = mybir.AluOpType
P = 128

PROBE_VALS = [-5.0, -10.0, -15.0, -20.0, -30.0, -40.0, -60.0, -80.0]


def build_nc(b_loc=4, s=2048, h=1024, n_cores=8,
             warm_a=10, enc_bufs=4, pe_bufs=6, eng_bufs=4, hp_at=5,
             sc=512):
    SC = sc                  # s-chunk width (512 = one PSUM bank of f32)
    n_sc = s // SC           # s-chunks per batch row
    n_hc = h // P            # contraction chunks
    n_ot = h // P            # output (o) tiles

    nc = bacc.Bacc("TRN2", target_bir_lowering=False, debug=False,
                   num_devices=n_cores)

    wt = nc.dram_tensor("wt", [2 * h, h], BF16, kind="ExternalInput").ap()
    hiddenT = nc.dram_tensor("hiddenT", [h, b_loc], BF16, kind="ExternalInput").ap()
    b_attn = nc.dram_tensor("b_attn", [h], F32, kind="ExternalInput").ap()
    v = nc.dram_tensor("v", [h], F32, kind="ExternalInput").ap()
    encT = nc.dram_tensor("encT", [b_loc, h, s], BF16, kind="ExternalInput").ap()
    probe = nc.dram_tensor("probe", [1, 8], F32, kind="ExternalInput").ap()
    nb = nc.dram_tensor("nb", [1, 1], F32, kind="ExternalInput").ap()
    out = nc.dram_tensor("out", [b_loc, s], F32, kind="ExternalOutput").ap()
    dbg = nc.dram_tensor("dbg", [1, 8], F32, kind="ExternalOutput").ap()

    wt_r = wt.rearrange("(jc p) o -> p jc o", p=P)

    with tile.TileContext(nc) as tc, ExitStack() as ctx:
        const = ctx.enter_context(tc.tile_pool(name="const", bufs=1))
        pe_p = ctx.enter_context(tc.tile_pool(name="pe", bufs=pe_bufs, space="PSUM"))
        pa_p = ctx.enter_context(tc.tile_pool(name="pa", bufs=2, space="PSUM"))
        encp = ctx.enter_context(tc.tile_pool(name="encp", bufs=enc_bufs))
        engp = ctx.enter_context(tc.tile_pool(name="engp", bufs=eng_bufs))
        accp = ctx.enter_context(tc.tile_pool(name="accp", bufs=2))

        # ---- zeros for warmup; ones column for the partition-reduce ----
        wz = const.tile([P, SC], BF16)
        nc.gpsimd.memset(wz[:], 0)
        # full 128x128 stationary operand (ones in column 0, zeros
        # elsewhere): the v-dot reduce then streams through the weight
        # pipeline exactly like the e_proj matmuls — an M=1 ones-column
        # breaks the LDWEIGHTS pull-ahead and costs ~0.6us per unit
        onesM = const.tile([P, P], BF16)
        nc.gpsimd.memset(onesM[:], 0)
        nc.gpsimd.memset(onesM[:, 0:1], 1.0)

        def warm(n):
            # independent back-to-back matmuls cycling the pe pool: a
            # gapless PE burst (a semaphore-serialized chain never trips
            # the HAM activity window — it needs contiguous busy time)
            for _ in range(n):
                pw = pe_p.tile([P, SC], F32, name="pe")
                nc.tensor.matmul(pw[:], wz[:, :P], wz[:], start=True, stop=True)

        warm(warm_a)

        # ---- small constants (scalar HWDGE queue) ----
        hT_bf = const.tile([P, n_hc, b_loc], BF16)
        nc.scalar.dma_start(hT_bf[:], hiddenT.rearrange("(hc p) b -> p hc b", p=P))
        baT = const.tile([P, n_ot], F32)
        nc.scalar.dma_start(baT[:], b_attn.rearrange("(oc p) -> p oc", p=P))
        vT = const.tile([P, n_ot], F32)
        nc.scalar.dma_start(vT[:], v.rearrange("(oc p) -> p oc", p=P))
        probe_t = const.tile([1, 8], F32)
        nc.scalar.dma_start(probe_t[:], probe)
        nb_t = const.tile([1, 1], F32)
        nc.scalar.dma_start(nb_t[:], nb)
        # Wh on the scalar HWDGE ring: streams in parallel with the
        # We/enc0 critical sequence on the sync ring

        # ---- critical-order sync queue: We chunks and unit-0 enc chunks
        # interleaved (everything the first matmul group needs lands
        # first), then Wh, then the remaining enc units ----
        wt_bf = const.tile([P, 2 * n_hc, h], BF16)
        it0 = encp.tile([P, n_hc, SC], BF16, name="it")
        enc0_r = encT[0, :, 0:SC].rearrange("(hc p) s -> p hc s", p=P)
        for cch in range(n_hc):
            nc.sync.dma_start(wt_bf[:, n_hc + cch, :], wt_r[:, n_hc + cch, :])
            # enc0 on the SWDGE path: separate descriptor engine, so the
            # sync HWDGE ring streams We at full descriptor rate
            nc.gpsimd.dma_start(it0[:, cch, :], enc0_r[:, cch, :])
        # Wh rides the scalar ring, in parallel with the critical
        # We/enc0 sequence on the sync ring; h_proj needs it only once
        # unit 0 reaches its hp splice point
        nc.scalar.dma_start(wt_bf[:, 0:n_hc, :], wt_r[:, 0:n_hc, :])

        e_rows = [const.tile([1, s], F32, name=f"e_r{i}")
                  for i in range(b_loc)]
        ssc = const.tile([1, b_loc * n_sc], F32)   # per-unit exp partial sums
        hb = const.tile([P, n_ot, b_loc], F32)

        units = [(b, c) for b in range(b_loc) for c in range(n_sc)]

        def load_unit(b, c):
            it = encp.tile([P, n_hc, SC], BF16, name="it")
            nc.sync.dma_start(
                it[:],
                encT[b, :, c * SC:(c + 1) * SC].rearrange(
                    "(hc p) s -> p hc s", p=P))
            return it

        def h_proj():
            # hb[:, ot, b] = (Wh.T chunk @ hT)[o, b] + b_attn[o]; the
            # bias-add rides the scalar engine (Identity + per-partition
            # bias) — it must precede every tanh in the ACT FIFO
            for ot in range(n_ot):
                # borrows the ones-matmul's PSUM bank (first real use
                # of that bank comes long after h_proj retires)
                pht = pa_p.tile([P, SC], F32, name="pa")
                ph = pht[:, 0:b_loc]
                for hc in range(n_hc):
                    nc.tensor.matmul(
                        ph, wt_bf[:, hc, ot * P:(ot + 1) * P],
                        hT_bf[:, hc, :],
                        start=(hc == 0), stop=(hc == n_hc - 1))
                nc.scalar.add(hb[:, ot, :], ph, baT[:, ot, None])

        # ---- exp-table probe (negligible; feeds a host-side check) ----
        dbg_t = const.tile([1, 8], F32)
        nc.scalar.activation(dbg_t[:], probe_t[:], AF.Exp)
        nc.gpsimd.dma_start(dbg, dbg_t[:])

        def emit_ones(pending):
            # att chunk = ones.T @ accb
            b, c, accb = pending
            pa = pa_p.tile([P, SC], F32, name="pa")
            nc.tensor.matmul(pa[:], onesM[:], accb[:],
                             start=True, stop=True)
            return (b, c, pa)

        def emit_exp(pending):
            # exp with the sum(|v|)-bound shift straight out of PSUM,
            # with a running per-chunk sum; deliberately emitted a full
            # unit late so it never head-of-line-blocks the next unit's
            # tanhs in the ACT FIFO while waiting on the ones-matmul
            b, c, pa = pending
            u = b * n_sc + c
            nc.scalar.activation(
                e_rows[b][:, c * SC:(c + 1) * SC], pa[0:1, :], AF.Exp,
                bias=nb_t[:], accum_out=ssc[:, u:u + 1])
            if c == n_sc - 1:
                softmax_b(b)

        def mm_group(it, ot):
            pe = pe_p.tile([P, SC], F32, name="pe")
            for hc in range(n_hc):
                nc.tensor.matmul(
                    pe[:], wt_bf[:, n_hc + hc, ot * P:(ot + 1) * P],
                    it[:, hc, :],
                    start=(hc == 0), stop=(hc == n_hc - 1))
            return pe

        def tanh_fma(b, pe, ot, acc, accb, pending):
            eng = engp.tile([P, SC], BF16, name="eng")
            nc.scalar.activation(eng[:], pe[:], AF.Tanh,
                                 bias=hb[:, ot, b:b + 1])
            if ot == 0:
                nc.vector.tensor_scalar(
                    acc[:], eng[:], vT[:, 0:1], None, ALU.mult)
                if state.get("exp") is not None:
                    emit_exp(state.pop("exp"))
            else:
                # v-dot of the previous unit lags two ot-groups so its
                # accumulator is long finished when the PE reaches it
                if ot == 2 and pending is not None:
                    state["exp"] = emit_ones(pending)
                nc.vector.scalar_tensor_tensor(
                    accb[:] if ot == n_ot - 1 else acc[:],
                    eng[:], vT[:, ot:ot + 1], acc[:],
                    ALU.mult, ALU.add)

        state = {}

        def run_unit(b, c, it, pending, hp=False):
            acc = accp.tile([P, SC], F32, name="acc")
            accb = accp.tile([P, SC], BF16, name="accb")
            if hp:
                # unit 0: run the first hp_at matmul groups WITHOUT their
                # tanhs (PSUM banks hold them), then h_proj, then drain.
                # Every hb producer thereby precedes every tanh in the
                # ACT FIFO, and the PE never waits on the Wh DMA.
                pes = [mm_group(it, ot) for ot in range(hp_at)]
                h_proj()
                for ot, pe in enumerate(pes):
                    tanh_fma(b, pe, ot, acc, accb, pending)
                start = hp_at
            else:
                start = 0
            for ot in range(start, n_ot):
                pe = mm_group(it, ot)
                tanh_fma(b, pe, ot, acc, accb, pending)
            return (b, c, accb)

        def softmax_b(b):
            # all on partition 0: total = sum of the 4 chunk sums, then
            # normalize in halves on two engines and store
            ssum = const.tile([1, 1], F32, name=f"ssum{b}")
            nc.vector.tensor_reduce(
                ssum[:], ssc[:, b * n_sc:(b + 1) * n_sc],
                mybir.AxisListType.X, ALU.add)
            rinv = const.tile([1, 1], F32, name=f"rinv{b}")
            nc.vector.reciprocal(rinv[:], ssum[:])
            cut = 1280
            nc.vector.tensor_scalar(
                e_rows[b][:, 0:cut], e_rows[b][:, 0:cut], rinv[:],
                None, ALU.mult)
            nc.scalar.mul(e_rows[b][:, cut:s], e_rows[b][:, cut:s],
                          rinv[:])
            nc.scalar.dma_start(out[b:b + 1, :], e_rows[b][:])

        loaded = {0: it0}
        pending = None
        for idx, (b, c) in enumerate(units):
            for j in range(idx + 1, min(idx + enc_bufs, len(units))):
                if j not in loaded:
                    loaded[j] = load_unit(*units[j])
            pending = run_unit(b, c, loaded.pop(idx), pending, hp=(idx == 0))
        if state.get("exp") is not None:
            emit_exp(state.pop("exp"))
        emit_exp(emit_ones(pending))

    nc.compile()
    return nc


def make_in_maps(hidden, encoder_outputs, W_attn, b_attn, v, n_cores=8):
    hidden = np.asarray(hidden, dtype=np.float32)
    encoder_outputs = np.asarray(encoder_outputs, dtype=np.float32)
    W_attn = np.asarray(W_attn, dtype=np.float32)
    b_attn = np.asarray(b_attn, dtype=np.float32)
    v = np.asarray(v, dtype=np.float32)

    b = encoder_outputs.shape[0]
    b_loc = b // n_cores
    wt = np.ascontiguousarray(W_attn.T.astype(ml_dtypes.bfloat16))
    probe = np.array([PROBE_VALS], dtype=np.float32)
    # |att| <= sum|v| since |tanh| <= 1; shifting exp by this bound is
    # exact in infinite precision and keeps exp in (0, 1]
    nb = np.array([[-(np.abs(v).sum() + 1.0)]], dtype=np.float32)
    in_maps = []
    for i in range(n_cores):
        bsl = slice(b_loc * i, b_loc * (i + 1))
        in_maps.append({
            "wt": wt,
            "hiddenT": np.ascontiguousarray(
                hidden[0, bsl].T.astype(ml_dtypes.bfloat16)),
            "b_attn": b_attn,
            "v": v,
            "encT": np.ascontiguousarray(
                encoder_outputs[bsl].transpose(0, 2, 1)
                .astype(ml_dtypes.bfloat16)),
            "probe": probe,
            "nb": nb,
        })
    return in_maps


_NC_CACHE = {}


def _get_nc():
    if "nc" not in _NC_CACHE:
        _NC_CACHE["nc"] = build_nc(b_loc=4, s=2048, h=1024, n_cores=8)
    return _NC_CACHE["nc"]


def kernel(hidden, encoder_outputs, W_attn, b_attn, v):
    from concourse.bass_utils import run_bass_kernel_spmd

    nc = _get_nc()
    in_maps = make_in_maps(hidden, encoder_outputs, W_attn, b_attn, v,
                           n_cores=8)
    res = run_bass_kernel_spmd(nc, in_maps, core_ids=list(range(8)))
    out = np.concatenate([np.asarray(res.results[i]["out"])
                          for i in range(8)], axis=0)
    return out.astype(np.float32)
